# revision 1
# baseline (speedup 1.0000x reference)
"""NNConv+GRU message-passing network (ConvGRU) on 8 Trainium2 NeuronCores.

Strategy (per spec sharding hint, adapted):
  - Edges are sharded by OWNER OF DST node (8 node ranges of 1024). Each core
    computes the edge MLP + per-edge matvec + scatter-add purely locally for
    its own nodes (scatter realized as a small matmul against a static 0/1
    selection matrix -> exact duplicate handling).
  - Node state h is node-sharded for the GRU and AllGathered once per conv
    layer so every core can gather h[src] for its edges (indirect DMA).
  - The per-edge weight matrices We = reshape(mlp1/mlp2 MLP(edge_attr)) are
    never materialized in DRAM: PE computes We tiles into PSUM, ACT evacuates
    to SBUF fp16, DVE does the per-edge matvec as broadcast-multiply +
    strided reduction.
  - Pooling = matmul against a host-prebuilt (1/cnt)-scaled one-hot matrix,
    followed by a 16KB AllReduce; the output MLP is replicated.

Self-contained: only needs numpy + the concourse/bass stack installed in the
container. All shapes hardcoded for this problem size.
"""
import numpy as np

DIM = 64
DEPTHS = 3
N_NODES = 8192
N_EDGES = 16384
N_GRAPHS = 64
NC = 8
NPC = N_NODES // NC   # 1024 nodes per core
P = 128

TRACE = False
LAST_EXEC_NS = None
LAST_RESULTS = None

_CACHE = {}


def _build(T, b2_zero):
    """Build the (shared) 8-core SPMD program. Per-core data arrives via inputs."""
    import concourse.mybir as mybir
    import concourse.tile as tile
    from concourse import bacc
    import concourse.bass as bass
    from concourse.masks import make_identity

    f32 = mybir.dt.float32
    f16 = mybir.dt.float16
    i32 = mybir.dt.int32
    AF = mybir.ActivationFunctionType
    OP = mybir.AluOpType
    EP = T * P  # padded edge count per core

    nc = bacc.Bacc("TRN2", target_bir_lowering=False, debug=False, num_devices=NC)

    def din(name, shape, dt=f32):
        return nc.dram_tensor(name, shape, dt, kind="ExternalInput")

    xT_d = din("xT", [40, NPC])
    eaT_d = din("eaT", [10, EP], f16)
    srcx_d = din("srcidx", [P, T], i32)
    S_d = din("S", [P, T * NPC], f16)
    pS_d = din("poolS", [NPC, N_GRAPHS])
    fc0_wT_d = din("fc0_wT", [40, 32])
    fc0_b_d = din("fc0_b", [32, 1])
    g0_wihT_d = din("g0_wihT", [32, 192])
    g0_brz_d = din("g0_brz", [128, 1])
    g0_bihn_d = din("g0_bihn", [64, 1])
    g0_bhhn_d = din("g0_bhhn", [64, 1])
    w2p_d = [din(f"w2p{d}", [128, 4096], f16) for d in range(DEPTHS)]
    m1wT_d = [din(f"m1wT{d}", [10, 128], f16) for d in range(DEPTHS)]
    m1b_d = [din(f"m1b{d}", [128, 1]) for d in range(DEPTHS)]
    root_d = [din(f"root{d}", [64, 64]) for d in range(DEPTHS)]
    convb_d = [din(f"convb{d}", [64, 1]) for d in range(DEPTHS)]
    wihT_d = [din(f"wihT{d}", [64, 192]) for d in range(DEPTHS)]
    whhT_d = [din(f"whhT{d}", [64, 192]) for d in range(DEPTHS)]
    brz_d = [din(f"brz{d}", [128, 1]) for d in range(DEPTHS)]
    bihn_d = [din(f"bihn{d}", [64, 1]) for d in range(DEPTHS)]
    bhhn_d = [din(f"bhhn{d}", [64, 1]) for d in range(DEPTHS)]
    b2bc_d = None if b2_zero else [din(f"b2bc{d}", [128, 4096], f16) for d in range(DEPTHS)]
    o0wT_d = din("o0wT", [64, 64])
    o0b_d = din("o0b", [64, 1])
    o1wT_d = din("o1wT", [64, 32])
    o1b_d = din("o1b", [32, 1])
    o2wT_d = din("o2wT", [32, 1])
    o2b_d = din("o2b", [1, 1])

    y_d = nc.dram_tensor("y", [1, N_GRAPHS], f32, kind="ExternalOutput")

    RG = [list(range(NC))]

    with tile.TileContext(nc) as tc:
        with (
            tc.tile_pool(name="const", bufs=1) as cp,
            tc.tile_pool(name="work", bufs=2) as wp,
            tc.tile_pool(name="edge", bufs=4) as ep,
            tc.tile_pool(name="pwe", bufs=2, space="PSUM") as pwe,
            tc.tile_pool(name="pagg", bufs=1, space="PSUM") as pagg,
            tc.tile_pool(name="ptp", bufs=2, space="PSUM") as ptp,
            tc.tile_pool(name="dram", bufs=1, space="DRAM") as dp,
        ):
            # ---------------- constants to SBUF ----------------
            def load(name, dram, shape, dt=f32, ap=None):
                t = cp.tile(shape, dt, name=name)
                nc.sync.dma_start(t[:], dram[:, :] if ap is None else ap)
                return t

            xT = load("xT_s", xT_d, [40, NPC])
            eaT = load("eaT_s", eaT_d, [10, EP], f16)
            srcx = load("srcx_s", srcx_d, [P, T], i32)
            S = load("S_s", S_d, [P, T * NPC], f16)
            pS = cp.tile([P, 8 * N_GRAPHS], f32, name="pS_s")
            for c in range(8):
                nc.sync.dma_start(
                    pS[:, c * N_GRAPHS:(c + 1) * N_GRAPHS],
                    pS_d[c * P:(c + 1) * P, :],
                )
            fc0_wT = load("fc0_wT_s", fc0_wT_d, [40, 32])
            fc0_b = load("fc0_b_s", fc0_b_d, [32, 1])
            g0_wihT = load("g0_wihT_s", g0_wihT_d, [32, 192])
            g0_brz = load("g0_brz_s", g0_brz_d, [128, 1])
            g0_bihn = load("g0_bihn_s", g0_bihn_d, [64, 1])
            g0_bhhn = load("g0_bhhn_s", g0_bhhn_d, [64, 1])
            w2p = [load(f"w2p_s{d}", w2p_d[d], [128, 4096], f16) for d in range(DEPTHS)]
            m1wT = [load(f"m1wT_s{d}", m1wT_d[d], [10, 128], f16) for d in range(DEPTHS)]
            m1b = [load(f"m1b_s{d}", m1b_d[d], [128, 1]) for d in range(DEPTHS)]
            rootw = [load(f"root_s{d}", root_d[d], [64, 64]) for d in range(DEPTHS)]
            convb = [load(f"convb_s{d}", convb_d[d], [64, 1]) for d in range(DEPTHS)]
            wihT = [load(f"wihT_s{d}", wihT_d[d], [64, 192]) for d in range(DEPTHS)]
            whhT = [load(f"whhT_s{d}", whhT_d[d], [64, 192]) for d in range(DEPTHS)]
            brz = [load(f"brz_s{d}", brz_d[d], [128, 1]) for d in range(DEPTHS)]
            bihn = [load(f"bihn_s{d}", bihn_d[d], [64, 1]) for d in range(DEPTHS)]
            bhhn = [load(f"bhhn_s{d}", bhhn_d[d], [64, 1]) for d in range(DEPTHS)]
            b2bc = (
                None if b2_zero else
                [load(f"b2bc_s{d}", b2bc_d[d], [128, 4096], f16) for d in range(DEPTHS)]
            )
            o0wT = load("o0wT_s", o0wT_d, [64, 64])
            o0b = load("o0b_s", o0b_d, [64, 1])
            o1wT = load("o1wT_s", o1wT_d, [64, 32])
            o1b = load("o1b_s", o1b_d, [32, 1])
            o2wT = load("o2wT_s", o2wT_d, [32, 1])
            o2b = load("o2b_s", o2b_d, [1, 1])

            ident = cp.tile([P, P], f32, name="ident")
            make_identity(nc, ident[:])

            hown = [dp.tile([NPC, DIM], f32, name=f"hown{d}") for d in range(DEPTHS)]
            hfull = [dp.tile([N_NODES, DIM], f32, name=f"hfull{d}") for d in range(DEPTHS)]
            ar_in = dp.tile([DIM, N_GRAPHS], f32, name="ar_in")
            ar_out = dp.tile([DIM, N_GRAPHS], f32, name="ar_out")

            # ---------------- helpers ----------------
            def mm512(out_ap_fn, lhsT, rhs_fn, n_total, start, stop):
                """matmuls in 512-wide chunks: out[:, s] = lhsT.T @ rhs[:, s]."""
                off = 0
                while off < n_total:
                    n = min(512, n_total - off)
                    nc.tensor.matmul(
                        out_ap_fn(off, n), lhsT, rhs_fn(off, n),
                        start=start, stop=stop,
                    )
                    off += n

            def gru_elem(d, rz_s, gi_n_s, hn_s, h_prev, tagp):
                """rz_s [128,1024] (r||z, post-sigmoid), gi_n_s/hn_s [64,1024].
                Returns new h_T [64,1024] f32: h' = n + z*(h - n)."""
                # DVE needs equal base partitions for SBUF+SBUF tensor_tensor,
                # so shift the z half down to a base-0 tile via SBUF->SBUF DMA.
                z_s = wp.tile([64, NPC], f32, name=f"z_{tagp}", tag="gru_z")
                nc.sync.dma_start(z_s[:], rz_s[64:128, :])
                t1 = wp.tile([64, NPC], f32, name=f"t1_{tagp}", tag="gru_t1")
                nc.vector.tensor_tensor(out=t1[:], in0=rz_s[0:64, :], in1=hn_s[:], op=OP.mult)
                nc.vector.tensor_tensor(out=t1[:], in0=t1[:], in1=gi_n_s[:], op=OP.add)
                nt = wp.tile([64, NPC], f32, name=f"nt_{tagp}", tag="gru_nt")
                nc.scalar.activation(nt[:], t1[:], AF.Tanh)
                hm = wp.tile([64, NPC], f32, name=f"hm_{tagp}", tag="gru_hm")
                if h_prev is None:
                    # h=0: h' = n - z*n
                    nc.vector.tensor_tensor(out=hm[:], in0=z_s[:], in1=nt[:], op=OP.mult)
                    hnew = wp.tile([64, NPC], f32, name=f"h_{tagp}", tag="hT")
                    nc.vector.tensor_tensor(out=hnew[:], in0=nt[:], in1=hm[:], op=OP.subtract)
                else:
                    nc.vector.tensor_tensor(out=hm[:], in0=h_prev[:], in1=nt[:], op=OP.subtract)
                    nc.vector.tensor_tensor(out=hm[:], in0=hm[:], in1=z_s[:], op=OP.mult)
                    hnew = wp.tile([64, NPC], f32, name=f"h_{tagp}", tag="hT")
                    nc.vector.tensor_tensor(out=hnew[:], in0=hm[:], in1=nt[:], op=OP.add)
                return hnew

            def h_to_node_major(h_T, tagp):
                """PE-transpose h_T [64,1024] -> 8 sbuf tiles [128,64] node-major."""
                outs = []
                for c in range(8):
                    tp = ptp.tile([P, DIM], f32, name=f"tp_{tagp}_{c}", tag="tp")
                    nc.tensor.transpose(
                        out=tp[:], in_=h_T[:, c * P:(c + 1) * P], identity=ident[0:64, 0:64]
                    )
                    hm = wp.tile([P, DIM], f32, name=f"hnm_{tagp}_{c}", tag=f"hnm{c}")
                    nc.scalar.activation(hm[:], tp[:], AF.Copy)
                    outs.append(hm)
                return outs

            # ---------------- phase 0: fc0 + gru0 (h0 = 0) ----------------
            x0_ps = pwe.tile([P, NPC], f32, name="x0_ps", tag="pwe")
            mm512(lambda o, n: x0_ps[0:32, o:o + n], fc0_wT[:],
                  lambda o, n: xT[:, o:o + n], NPC, True, True)
            x0r = wp.tile([32, NPC], f32, name="x0r")
            nc.scalar.activation(x0r[:], x0_ps[0:32, :], AF.Relu, bias=fc0_b[:, 0:1])

            g0rz_ps = pwe.tile([P, NPC], f32, name="g0rz_ps", tag="pwe")
            mm512(lambda o, n: g0rz_ps[0:128, o:o + n], g0_wihT[:, 0:128],
                  lambda o, n: x0r[:, o:o + n], NPC, True, True)
            rz0 = wp.tile([P, NPC], f32, name="rz0", tag="gru_rz")
            nc.scalar.activation(rz0[:], g0rz_ps[0:128, :], AF.Sigmoid, bias=g0_brz[:, 0:1])

            g0n_ps = pwe.tile([P, NPC], f32, name="g0n_ps", tag="pwe")
            mm512(lambda o, n: g0n_ps[0:64, o:o + n], g0_wihT[:, 128:192],
                  lambda o, n: x0r[:, o:o + n], NPC, True, True)
            gin0 = wp.tile([64, NPC], f32, name="gin0", tag="gru_gin")
            nc.scalar.activation(gin0[:], g0n_ps[0:64, :], AF.Identity, bias=g0_bihn[:, 0:1])
            # hn = r * bhh_n  (h=0 so gh_n = bhh_n, broadcast per partition)
            hn0 = wp.tile([64, NPC], f32, name="hn0", tag="gru_hn")
            nc.vector.tensor_scalar_mul(hn0[:], rz0[0:64, :], g0_bhhn[:, 0:1])
            # n = tanh(gi_n + bih_n + r*bhh_n): gi_n_s already has bih_n; hn0 = r*bhh_n
            h_T = gru_elem(-1, rz0, gin0, hn0, None, "p0")

            h_nm = h_to_node_major(h_T, "p0")
            for c in range(8):
                nc.sync.dma_start(hown[0][c * P:(c + 1) * P, :], h_nm[c][:])
            nc.gpsimd.collective_compute(
                "AllGather", OP.bypass, replica_groups=RG,
                ins=[hown[0].opt()], outs=[hfull[0].opt()],
            )

            # ---------------- conv depths ----------------
            for d in range(DEPTHS):
                # edge-MLP hidden: hidT [128, EP] fp16 (k on partitions)
                hidT = wp.tile([P, EP], f16, name=f"hidT{d}", tag="hidT")
                off = 0
                while off < EP:
                    n = min(1024, EP - off)
                    hp = pwe.tile([P, NPC], f32, name=f"hid_ps{d}_{off}", tag="pwe")
                    mm512(lambda o, nn, _b=off: hp[:, o:o + nn], m1wT[d][:],
                          lambda o, nn, _b=off: eaT[:, _b + o:_b + o + nn], n, True, True)
                    nc.scalar.activation(
                        hidT[:, off:off + n], hp[:, 0:n], AF.Relu, bias=m1b[d][:, 0:1]
                    )
                    off += n

                aggT = pagg.tile([64, NPC], f32, name=f"aggT{d}", tag="agg")

                for t in range(T):
                    hsf = ep.tile([P, DIM], f32, name=f"hsf{d}_{t}", tag="hsf")
                    nc.gpsimd.indirect_dma_start(
                        out=hsf[:], out_offset=None,
                        in_=hfull[d][:, :],
                        in_offset=bass.IndirectOffsetOnAxis(ap=srcx[:, t:t + 1], axis=0),
                    )
                    hsb = ep.tile([P, DIM], f16, name=f"hsb{d}_{t}", tag="hsb")
                    nc.vector.tensor_copy(hsb[:], hsf[:])

                    msg = ep.tile([P, DIM], f32, name=f"msg{d}_{t}", tag="msg")
                    for q in range(4):
                        wps = pwe.tile([P, NPC], f32, name=f"we{d}_{t}_{q}", tag="pwe")
                        mm512(lambda o, n, _q=q, _t=t: wps[:, o:o + n],
                              hidT[:, t * P:(t + 1) * P],
                              lambda o, n, _q=q: w2p[d][:, _q * 1024 + o:_q * 1024 + o + n],
                              1024, True, True)
                        wsb = ep.tile([P, 1024], f16, name=f"wsb{d}_{t}_{q}", tag="wsb")
                        nc.scalar.activation(wsb[:], wps[:], AF.Copy)
                        if b2bc is not None:
                            nc.vector.tensor_tensor(
                                out=wsb[:], in0=wsb[:],
                                in1=b2bc[d][:, q * 1024:(q + 1) * 1024], op=OP.add,
                            )
                        prod = ep.tile([P, 1024], f16, name=f"prod{d}_{t}_{q}", tag="prod")
                        nc.vector.tensor_tensor(
                            out=prod[:].rearrange("p (o i) -> p o i", i=64),
                            in0=wsb[:].rearrange("p (o i) -> p o i", i=64),
                            in1=hsb[:, :].unsqueeze(1).to_broadcast([P, 16, 64]),
                            op=OP.mult,
                        )
                        nc.vector.tensor_reduce(
                            out=msg[:, q * 16:(q + 1) * 16],
                            in_=prod[:].rearrange("p (o i) -> p o i", i=64),
                            axis=mybir.AxisListType.X,
                            op=OP.add,
                        )
                    msgh = ep.tile([P, DIM], f16, name=f"msgh{d}_{t}", tag="msgh")
                    nc.vector.tensor_copy(msgh[:], msg[:])
                    # scatter-add into aggT via selection matmul
                    for s in range(2):
                        nc.tensor.matmul(
                            aggT[0:64, s * 512:(s + 1) * 512],
                            msgh[:],
                            S[:, t * NPC + s * 512: t * NPC + (s + 1) * 512],
                            start=(t == 0), stop=False,
                        )
                # + h @ root_w  (accumulate into aggT, fp32)
                for s in range(2):
                    nc.tensor.matmul(
                        aggT[0:64, s * 512:(s + 1) * 512],
                        rootw[d][:],
                        h_T[:, s * 512:(s + 1) * 512],
                        start=False, stop=True,
                    )
                xc = wp.tile([64, NPC], f32, name=f"xc{d}", tag="xc")
                nc.scalar.activation(xc[:], aggT[0:64, :], AF.Relu, bias=convb[d][:, 0:1])

                # ---- GRU(xc, h) ----
                gi_ps = pwe.tile([P, NPC], f32, name=f"girz{d}", tag="pwe")
                mm512(lambda o, n: gi_ps[0:128, o:o + n], wihT[d][:, 0:128],
                      lambda o, n: xc[:, o:o + n], NPC, True, True)
                girz = wp.tile([P, NPC], f32, name=f"girzs{d}", tag="gru_girz")
                nc.scalar.activation(girz[:], gi_ps[0:128, :], AF.Copy)

                gh_ps = pwe.tile([P, NPC], f32, name=f"ghrz{d}", tag="pwe")
                mm512(lambda o, n: gh_ps[0:128, o:o + n], whhT[d][:, 0:128],
                      lambda o, n: h_T[:, o:o + n], NPC, True, True)
                rzs = wp.tile([P, NPC], f32, name=f"rzs{d}", tag="gru_rzs")
                nc.vector.tensor_tensor(out=rzs[:], in0=girz[:], in1=gh_ps[0:128, :], op=OP.add)
                rz = wp.tile([P, NPC], f32, name=f"rz{d}", tag="gru_rz")
                nc.scalar.activation(rz[:], rzs[:], AF.Sigmoid, bias=brz[d][:, 0:1])

                gin_ps = pwe.tile([P, NPC], f32, name=f"gin{d}", tag="pwe")
                mm512(lambda o, n: gin_ps[0:64, o:o + n], wihT[d][:, 128:192],
                      lambda o, n: xc[:, o:o + n], NPC, True, True)
                gin = wp.tile([64, NPC], f32, name=f"gins{d}", tag="gru_gin")
                nc.scalar.activation(gin[:], gin_ps[0:64, :], AF.Identity, bias=bihn[d][:, 0:1])

                ghn_ps = pwe.tile([P, NPC], f32, name=f"ghn{d}", tag="pwe")
                mm512(lambda o, n: ghn_ps[0:64, o:o + n], whhT[d][:, 128:192],
                      lambda o, n: h_T[:, o:o + n], NPC, True, True)
                hn = wp.tile([64, NPC], f32, name=f"hns{d}", tag="gru_hn")
                nc.scalar.activation(hn[:], ghn_ps[0:64, :], AF.Identity, bias=bhhn[d][:, 0:1])
                # n = tanh(gi_n + bih_n + r*(gh_n + bhh_n))
                h_T = gru_elem(d, rz, gin, hn, h_T, f"d{d}")

                h_nm = h_to_node_major(h_T, f"d{d}")
                if d < DEPTHS - 1:
                    for c in range(8):
                        nc.sync.dma_start(hown[d + 1][c * P:(c + 1) * P, :], h_nm[c][:])
                    nc.gpsimd.collective_compute(
                        "AllGather", OP.bypass, replica_groups=RG,
                        ins=[hown[d + 1].opt()], outs=[hfull[d + 1].opt()],
                    )
                else:
                    pooled_ps = ptp.tile([64, N_GRAPHS], f32, name="pooled_ps", tag="tp")
                    for c in range(8):
                        nc.tensor.matmul(
                            pooled_ps[0:64, :],
                            h_nm[c][:],
                            pS[:, c * N_GRAPHS:(c + 1) * N_GRAPHS],
                            start=(c == 0), stop=(c == 7),
                        )
                    pooled_sb = wp.tile([64, N_GRAPHS], f32, name="pooled_sb")
                    nc.scalar.activation(pooled_sb[:], pooled_ps[0:64, :], AF.Copy)
                    nc.sync.dma_start(ar_in[:, :], pooled_sb[:])

            # ---------------- pooling AllReduce + output MLP ----------------
            nc.gpsimd.collective_compute(
                "AllReduce", OP.add, replica_groups=RG,
                ins=[ar_in.opt()], outs=[ar_out.opt()],
            )
            pooled = wp.tile([64, N_GRAPHS], f32, name="pooled")
            nc.sync.dma_start(pooled[:], ar_out[:, :])

            m1_ps = ptp.tile([64, N_GRAPHS], f32, name="m1_ps", tag="tp")
            nc.tensor.matmul(m1_ps[0:64, :], o0wT[:], pooled[:], start=True, stop=True)
            m1r = wp.tile([64, N_GRAPHS], f32, name="m1r")
            nc.scalar.activation(m1r[:], m1_ps[0:64, :], AF.Relu, bias=o0b[:, 0:1])

            m2_ps = ptp.tile([64, N_GRAPHS], f32, name="m2_ps", tag="tp")
            nc.tensor.matmul(m2_ps[0:32, :], o1wT[:], m1r[:], start=True, stop=True)
            m2b = wp.tile([32, N_GRAPHS], f32, name="m2b")
            nc.scalar.activation(m2b[:], m2_ps[0:32, :], AF.Identity, bias=o1b[:, 0:1])

            m3_ps = ptp.tile([64, N_GRAPHS], f32, name="m3_ps", tag="tp")
            nc.tensor.matmul(m3_ps[0:1, :], o2wT[:], m2b[:], start=True, stop=True)
            ysb = wp.tile([1, N_GRAPHS], f32, name="ysb")
            nc.scalar.activation(ysb[:], m3_ps[0:1, :], AF.Identity, bias=o2b[:, 0:1])
            nc.sync.dma_start(y_d[:, :], ysb[:])

    nc.finalize()
    return nc


def _prep(inputs):
    """Host-side sharding + weight permutation. Returns (T, b2_zero, in_maps)."""
    g = lambda k: np.asarray(inputs[k])
    x = g("x").astype(np.float32)
    ea = g("edge_attr").astype(np.float32)
    ei = g("edge_index").astype(np.int64)
    batch = g("batch").astype(np.int64)
    src, dst = ei[0], ei[1]

    owner = dst // NPC
    core_ids = [np.nonzero(owner == c)[0] for c in range(NC)]
    T = int(max((len(ids) + P - 1) // P for ids in core_ids))
    T = max(T, 1)
    EP = T * P

    cnt = np.bincount(batch, minlength=N_GRAPHS).astype(np.float32)
    inv = 1.0 / np.maximum(cnt, 1.0)

    mlp2_b = g("mlp2_b").astype(np.float32)
    b2_zero = bool(np.all(mlp2_b == 0))

    # ---- shared weights
    shared = {
        "fc0_wT": g("fc0_w").astype(np.float32).T.copy(),
        "fc0_b": g("fc0_b").astype(np.float32)[:, None],
        "g0_wihT": g("gru0_wih").astype(np.float32).T.copy(),
        "g0_brz": (g("gru0_bih") + g("gru0_bhh")).astype(np.float32)[:128, None],
        "g0_bihn": g("gru0_bih").astype(np.float32)[128:, None],
        "g0_bhhn": g("gru0_bhh").astype(np.float32)[128:, None],
        "o0wT": g("out0_w").astype(np.float32).T.copy(),
        "o0b": g("out0_b").astype(np.float32)[:, None],
        "o1wT": g("out1_w").astype(np.float32).T.copy(),
        "o1b": g("out1_b").astype(np.float32)[:, None],
        "o2wT": g("out2_w").astype(np.float32).T.copy(),
        "o2b": g("out2_b").astype(np.float32)[:, None],
    }
    mlp1_w = g("mlp1_w").astype(np.float32)
    mlp1_b = g("mlp1_b").astype(np.float32)
    mlp2_w = g("mlp2_w").astype(np.float32)
    root_w = g("root_w").astype(np.float32)
    conv_b = g("conv_b").astype(np.float32)
    gru_wih = g("gru_wih").astype(np.float32)
    gru_whh = g("gru_whh").astype(np.float32)
    gru_bih = g("gru_bih").astype(np.float32)
    gru_bhh = g("gru_bhh").astype(np.float32)
    for d in range(DEPTHS):
        shared[f"w2p{d}"] = (
            mlp2_w[d].reshape(64, 64, 128).transpose(2, 1, 0).reshape(128, 4096)
        ).astype(np.float16)
        shared[f"m1wT{d}"] = mlp1_w[d].T.astype(np.float16).copy()
        shared[f"m1b{d}"] = mlp1_b[d][:, None].copy()
        shared[f"root{d}"] = root_w[d].copy()
        shared[f"convb{d}"] = conv_b[d][:, None].copy()
        shared[f"wihT{d}"] = gru_wih[d].T.copy()
        shared[f"whhT{d}"] = gru_whh[d].T.copy()
        shared[f"brz{d}"] = (gru_bih[d] + gru_bhh[d])[:128, None].copy()
        shared[f"bihn{d}"] = gru_bih[d][128:, None].copy()
        shared[f"bhhn{d}"] = gru_bhh[d][128:, None].copy()
        if not b2_zero:
            b2p = mlp2_b[d].reshape(64, 64).T.reshape(4096)  # [(o,i)]
            shared[f"b2bc{d}"] = np.broadcast_to(
                b2p.astype(np.float16), (P, 4096)
            ).copy()

    in_maps = []
    for c in range(NC):
        ids = core_ids[c]
        n_real = len(ids)
        src_pad = np.zeros(EP, np.int32)
        src_pad[:n_real] = src[ids]
        ea_pad = np.zeros((EP, 10), np.float32)
        ea_pad[:n_real] = ea[ids]
        S_full = np.zeros((EP, NPC), np.float16)
        S_full[np.arange(n_real), dst[ids] - c * NPC] = 1.0
        S_tab = np.zeros((P, T * NPC), np.float16)
        for t in range(T):
            S_tab[:, t * NPC:(t + 1) * NPC] = S_full[t * P:(t + 1) * P]
        pm = np.zeros((NPC, N_GRAPHS), np.float32)
        nb = batch[c * NPC:(c + 1) * NPC]
        pm[np.arange(NPC), nb] = inv[nb]
        m = {
            "xT": x[c * NPC:(c + 1) * NPC].T.copy(),
            "eaT": ea_pad.T.astype(np.float16).copy(),
            "srcidx": src_pad.reshape(T, P).T.copy(),
            "S": S_tab,
            "poolS": pm,
        }
        m.update(shared)
        in_maps.append(m)
    return T, b2_zero, in_maps


def kernel(**inputs) -> np.ndarray:
    global LAST_EXEC_NS, LAST_RESULTS
    T, b2_zero, in_maps = _prep(inputs)
    key = (T, b2_zero)
    if key not in _CACHE:
        _CACHE[key] = _build(T, b2_zero)
    nc = _CACHE[key]

    from concourse.bass_utils import run_bass_kernel_spmd

    if TRACE:
        res = run_bass_kernel_spmd(
            nc, in_maps, list(range(NC)), trace=True, trace_cores=list(range(NC))
        )
        LAST_EXEC_NS = res.exec_time_ns
        LAST_RESULTS = res
    else:
        res = run_bass_kernel_spmd(nc, in_maps, list(range(NC)))
    return res.results[0]["y"].reshape(N_GRAPHS).astype(np.float32)



# revision 6
# speedup vs baseline: 1.0748x; 1.0748x over previous
"""NNConv+GRU message-passing network (ConvGRU) on 8 Trainium2 NeuronCores.

Strategy (v2):
  - Edges sharded by OWNER OF DST node (8 ranges of 1024 nodes). Each core
    computes edge MLP + per-edge matvec + scatter-add locally; scatter is a
    matmul against a static 0/1 selection matrix (exact duplicate handling).
  - Node state h node-sharded for the GRU; h exchanged across cores in f16
    via AllGather once per conv layer; per-edge h[src] gathered by indirect
    DMA directly in f16.
  - All matmuls run in fp16 (fp32 matmuls cost 4 cyc/col on TRN2 PE).
  - Per-edge weight matrices We never touch DRAM: PE -> PSUM f32, ACT
    evacuates to SBUF f16, DVE does one broadcast-multiply per tile (2x_1p
    fast mode) and a packed-halves ADD TREE (tensor_tensor 2x) instead of
    tensor_reduce (which has no DVE fast modes); two mid tree levels run on
    GpSimd to balance engine load.
  - Scatter matmuls are deferred to the end of each depth so PE streams all
    We matmuls back to back (p-state ramp, no per-tile PE<->DVE lockstep).
  - GRU r/z gates computed by ONE stacked matmul over [xc; h] (k=128).

Self-contained: numpy + concourse/bass only. Shapes hardcoded.
"""
import numpy as np

DIM = 64
DEPTHS = 3
N_NODES = 8192
N_EDGES = 16384
N_GRAPHS = 64
NC = 8
NPC = N_NODES // NC   # 1024 nodes per core
P = 128

TRACE = False
LAST_EXEC_NS = None
LAST_RESULTS = None

_CACHE = {}


def _build(T, b2_zero):
    import concourse.mybir as mybir
    import concourse.tile as tile
    from concourse import bacc
    import concourse.bass as bass
    from concourse.masks import make_identity

    f32 = mybir.dt.float32
    f16 = mybir.dt.float16
    i32 = mybir.dt.int32
    AF = mybir.ActivationFunctionType
    OP = mybir.AluOpType
    EP = T * P

    nc = bacc.Bacc("TRN2", target_bir_lowering=False, debug=False, num_devices=NC)

    def din(name, shape, dt=f32):
        return nc.dram_tensor(name, shape, dt, kind="ExternalInput")

    xT_d = din("xT", [40, NPC], f16)
    eaT_d = din("eaT", [10, EP], f16)
    srcx_d = din("srcidx", [P, T], i32)
    S_d = din("S", [P, T * NPC], f16)
    pS_d = din("poolS", [NPC, N_GRAPHS], f16)
    fc0_wT_d = din("fc0_wT", [40, 32], f16)
    fc0_b_d = din("fc0_b", [32, 1])
    g0_wihT_d = din("g0_wihT", [32, 192], f16)
    g0_brz_d = din("g0_brz", [128, 1])
    g0_bihn_d = din("g0_bihn", [64, 1])
    g0_bhhn_d = din("g0_bhhn", [64, 1])
    w2p_d = [din(f"w2p{d}", [128, 4096], f16) for d in range(DEPTHS)]
    m1wT_d = [din(f"m1wT{d}", [10, 128], f16) for d in range(DEPTHS)]
    m1b_d = [din(f"m1b{d}", [128, 1]) for d in range(DEPTHS)]
    root_d = [din(f"root{d}", [64, 64], f16) for d in range(DEPTHS)]
    convb_d = [din(f"convb{d}", [64, 1]) for d in range(DEPTHS)]
    grzT_d = [din(f"grzT{d}", [128, 128], f16) for d in range(DEPTHS)]  # [xc;h] -> r||z
    wnx_d = [din(f"wnx{d}", [64, 64], f16) for d in range(DEPTHS)]      # wih n-gate
    wnh_d = [din(f"wnh{d}", [64, 64], f16) for d in range(DEPTHS)]      # whh n-gate
    brz_d = [din(f"brz{d}", [128, 1]) for d in range(DEPTHS)]
    bihn_d = [din(f"bihn{d}", [64, 1]) for d in range(DEPTHS)]
    bhhn_d = [din(f"bhhn{d}", [64, 1]) for d in range(DEPTHS)]
    b2bc_d = None if b2_zero else [din(f"b2bc{d}", [128, 4096], f16) for d in range(DEPTHS)]
    o0wT_d = din("o0wT", [64, 64], f16)
    o0b_d = din("o0b", [64, 1])
    o1wT_d = din("o1wT", [64, 32], f16)
    o1b_d = din("o1b", [32, 1])
    o2wT_d = din("o2wT", [32, 1], f16)
    o2b_d = din("o2b", [1, 1])

    y_d = nc.dram_tensor("y", [1, N_GRAPHS], f32, kind="ExternalOutput")

    RG = [list(range(NC))]

    with tile.TileContext(nc) as tc:
        with (
            tc.tile_pool(name="const", bufs=1) as cp,
            tc.tile_pool(name="work", bufs=1) as wp,      # gru work tiles
            tc.tile_pool(name="state", bufs=2) as sp,     # h state tiles
            tc.tile_pool(name="edge", bufs=2) as ep,      # big per-tile buffers
            tc.tile_pool(name="gath", bufs=6) as gp,      # gathered h[src]
            tc.tile_pool(name="msgp", bufs=T + 1) as mp,  # per-tile msg outputs
            tc.tile_pool(name="pwe", bufs=2, space="PSUM") as pwe,
            tc.tile_pool(name="pagg", bufs=1, space="PSUM") as pagg,
            tc.tile_pool(name="ptp", bufs=2, space="PSUM") as ptp,
            tc.tile_pool(name="dram", bufs=1, space="DRAM") as dp,
        ):
            # ---------------- constants to SBUF ----------------
            def load(name, dram, shape, dt=f32, ap=None):
                t = cp.tile(shape, dt, name=name)
                nc.sync.dma_start(t[:], dram[:, :] if ap is None else ap)
                return t

            xT = load("xT_s", xT_d, [40, NPC], f16)
            eaT = load("eaT_s", eaT_d, [10, EP], f16)
            srcx = load("srcx_s", srcx_d, [P, T], i32)
            S = load("S_s", S_d, [P, T * NPC], f16)
            pS = cp.tile([P, 8 * N_GRAPHS], f16, name="pS_s")
            for c in range(8):
                nc.sync.dma_start(
                    pS[:, c * N_GRAPHS:(c + 1) * N_GRAPHS],
                    pS_d[c * P:(c + 1) * P, :],
                )
            fc0_wT = load("fc0_wT_s", fc0_wT_d, [40, 32], f16)
            fc0_b = load("fc0_b_s", fc0_b_d, [32, 1])
            g0_wihT = load("g0_wihT_s", g0_wihT_d, [32, 192], f16)
            g0_brz = load("g0_brz_s", g0_brz_d, [128, 1])
            g0_bihn = load("g0_bihn_s", g0_bihn_d, [64, 1])
            g0_bhhn = load("g0_bhhn_s", g0_bhhn_d, [64, 1])
            w2p = [load(f"w2p_s{d}", w2p_d[d], [128, 4096], f16) for d in range(DEPTHS)]
            m1wT = [load(f"m1wT_s{d}", m1wT_d[d], [10, 128], f16) for d in range(DEPTHS)]
            m1b = [load(f"m1b_s{d}", m1b_d[d], [128, 1]) for d in range(DEPTHS)]
            rootw = [load(f"root_s{d}", root_d[d], [64, 64], f16) for d in range(DEPTHS)]
            convb = [load(f"convb_s{d}", convb_d[d], [64, 1]) for d in range(DEPTHS)]
            grzT = [load(f"grzT_s{d}", grzT_d[d], [128, 128], f16) for d in range(DEPTHS)]
            wnx = [load(f"wnx_s{d}", wnx_d[d], [64, 64], f16) for d in range(DEPTHS)]
            wnh = [load(f"wnh_s{d}", wnh_d[d], [64, 64], f16) for d in range(DEPTHS)]
            brz = [load(f"brz_s{d}", brz_d[d], [128, 1]) for d in range(DEPTHS)]
            bihn = [load(f"bihn_s{d}", bihn_d[d], [64, 1]) for d in range(DEPTHS)]
            bhhn = [load(f"bhhn_s{d}", bhhn_d[d], [64, 1]) for d in range(DEPTHS)]
            b2bc = (
                None if b2_zero else
                [load(f"b2bc_s{d}", b2bc_d[d], [128, 4096], f16) for d in range(DEPTHS)]
            )
            o0wT = load("o0wT_s", o0wT_d, [64, 64], f16)
            o0b = load("o0b_s", o0b_d, [64, 1])
            o1wT = load("o1wT_s", o1wT_d, [64, 32], f16)
            o1b = load("o1b_s", o1b_d, [32, 1])
            o2wT = load("o2wT_s", o2wT_d, [32, 1], f16)
            o2b = load("o2b_s", o2b_d, [1, 1])

            ident = cp.tile([P, P], f32, name="ident")
            make_identity(nc, ident[:])

            hown = [dp.tile([NPC, DIM], f16, name=f"hown{d}") for d in range(DEPTHS)]
            hfull = [dp.tile([N_NODES, DIM], f16, name=f"hfull{d}") for d in range(DEPTHS)]
            ar_in = dp.tile([DIM, N_GRAPHS], f32, name="ar_in")
            ar_out = dp.tile([DIM, N_GRAPHS], f32, name="ar_out")

            # ---------------- helpers ----------------
            def mm512(out_ap_fn, lhsT, rhs_fn, n_total, start, stop):
                off = 0
                while off < n_total:
                    n = min(512, n_total - off)
                    nc.tensor.matmul(
                        out_ap_fn(off, n), lhsT, rhs_fn(off, n),
                        start=start, stop=stop,
                    )
                    off += n

            def boundary(d_next, h_T, hT16):
                """h_T f32 [64,1024] -> node-major f16 tiles, DRAM store,
                AllGather into hfull[d_next]; also fills xch[d] bottom half
                lazily via the hT16 tile the caller made."""
                outs = []
                for c in range(8):
                    tp = ptp.tile([P, DIM], f32, name=f"tp_b{d_next}_{c}", tag="tp")
                    nc.tensor.transpose(
                        out=tp[0:P, 0:DIM], in_=h_T[:, c * P:(c + 1) * P],
                        identity=ident[0:64, 0:64],
                    )
                    hm = wp.tile([P, DIM], f16, name=f"hnm_{d_next}_{c}", tag=f"hnm{c}")
                    nc.scalar.activation(hm[:], tp[:], AF.Copy)
                    outs.append(hm)
                for c in range(8):
                    nc.sync.dma_start(hown[d_next][c * P:(c + 1) * P, :], outs[c][:])
                nc.gpsimd.collective_compute(
                    "AllGather", OP.bypass, replica_groups=RG,
                    ins=[hown[d_next].opt()], outs=[hfull[d_next].opt()],
                )
                return outs

            # ---------------- phase 0: fc0 + gru0 (h0 = 0) ----------------
            x0_ps = pwe.tile([P, NPC], f32, name="x0_ps", tag="pwe")
            mm512(lambda o, n: x0_ps[0:32, o:o + n], fc0_wT[:],
                  lambda o, n: xT[:, o:o + n], NPC, True, True)
            x0r = wp.tile([32, NPC], f16, name="x0r")
            nc.scalar.activation(x0r[:], x0_ps[0:32, :], AF.Relu, bias=fc0_b[:, 0:1])

            # edge-MLP hidden for ALL depths up front (h-independent)
            hidT = []
            for d in range(DEPTHS):
                ht = cp.tile([P, EP], f16, name=f"hidT{d}")
                off = 0
                while off < EP:
                    n = min(1024, EP - off)
                    hp = pwe.tile([P, NPC], f32, name=f"hid_ps{d}_{off}", tag="pwe")
                    mm512(lambda o, nn, _b=off: hp[:, o:o + nn], m1wT[d][:],
                          lambda o, nn, _b=off: eaT[:, _b + o:_b + o + nn], n, True, True)
                    nc.scalar.activation(
                        ht[:, off:off + n], hp[:, 0:n], AF.Relu, bias=m1b[d][:, 0:1]
                    )
                    off += n
                hidT.append(ht)

            g0rz_ps = pwe.tile([P, NPC], f32, name="g0rz_ps", tag="pwe")
            mm512(lambda o, n: g0rz_ps[0:128, o:o + n], g0_wihT[:, 0:128],
                  lambda o, n: x0r[:, o:o + n], NPC, True, True)
            r0 = wp.tile([64, NPC], f32, name="r0", tag="gru_r")
            nc.scalar.activation(r0[:], g0rz_ps[0:64, :], AF.Sigmoid, bias=g0_brz[0:64, 0:1])
            z0 = wp.tile([64, NPC], f32, name="z0", tag="gru_z")
            nc.scalar.activation(z0[:], g0rz_ps[64:128, :], AF.Sigmoid, bias=g0_brz[64:128, 0:1])

            g0n_ps = pwe.tile([P, NPC], f32, name="g0n_ps", tag="pwe")
            mm512(lambda o, n: g0n_ps[0:64, o:o + n], g0_wihT[:, 128:192],
                  lambda o, n: x0r[:, o:o + n], NPC, True, True)
            gin0 = wp.tile([64, NPC], f32, name="gin0", tag="gru_gin")
            nc.scalar.activation(gin0[:], g0n_ps[0:64, :], AF.Identity, bias=g0_bihn[:, 0:1])

            # n = tanh(gi_n + bih_n + r*bhh_n); h' = n - z*n   (h0 = 0)
            t10 = wp.tile([64, NPC], f32, name="t10", tag="gru_t1")
            nc.vector.tensor_scalar_mul(t10[:], r0[:], g0_bhhn[:, 0:1])
            nc.vector.tensor_tensor(out=t10[:], in0=t10[:], in1=gin0[:], op=OP.add)
            nt0 = wp.tile([64, NPC], f32, name="nt0", tag="gru_nt")
            nc.scalar.activation(nt0[:], t10[:], AF.Tanh)
            hm0 = wp.tile([64, NPC], f32, name="hm0", tag="gru_hm")
            nc.vector.tensor_tensor(out=hm0[:], in0=z0[:], in1=nt0[:], op=OP.mult)
            h_T = sp.tile([64, NPC], f32, name="h_p0", tag="hT")
            nc.vector.tensor_tensor(out=h_T[:], in0=nt0[:], in1=hm0[:], op=OP.subtract)

            hT16 = sp.tile([64, NPC], f16, name="hT16_p0", tag="hT16")
            nc.vector.tensor_copy(hT16[:], h_T[:])
            boundary(0, h_T, hT16)

            # ---------------- conv depths ----------------
            for d in range(DEPTHS):
                # gathers first (gpsimd queue: AG(d) already issued before these)
                hsfs = []
                for t in range(T):
                    hsf = gp.tile([P, DIM], f16, name=f"hsf{d}_{t}", tag="hsf")
                    nc.gpsimd.indirect_dma_start(
                        out=hsf[:], out_offset=None,
                        in_=hfull[d][:, :],
                        in_offset=bass.IndirectOffsetOnAxis(ap=srcx[:, t:t + 1], axis=0),
                    )
                    hsfs.append(hsf)

                # xch: [xc ; h] stacked moving operand for GRU matmuls
                xch = sp.tile([P, NPC], f16, name=f"xch{d}", tag="xch")
                nc.sync.dma_start(xch[64:128, :], hT16[:])

                msgs = []
                for t in range(T):
                    wsb = ep.tile([P, 4096], f16, name=f"wsb{d}_{t}", tag="wsb")
                    for q in range(4):
                        wps = pwe.tile([P, NPC], f32, name=f"we{d}_{t}_{q}", tag="pwe")
                        mm512(lambda o, n, _q=q: wps[:, o:o + n],
                              hidT[d][:, t * P:(t + 1) * P],
                              lambda o, n, _q=q: w2p[d][:, _q * 1024 + o:_q * 1024 + o + n],
                              1024, True, True)
                        nc.scalar.activation(
                            wsb[:, q * 1024:(q + 1) * 1024], wps[:], AF.Copy
                        )
                    if b2bc is not None:
                        nc.vector.tensor_tensor(
                            out=wsb[:], in0=wsb[:], in1=b2bc[d][:], op=OP.add
                        )
                    # prod[e, (o,i)] = wsb * hsrc[e,i] ; one whole-tile DVE op
                    prod = ep.tile([P, 4096], f16, name=f"prod{d}_{t}", tag="prod")
                    nc.vector.tensor_tensor(
                        out=prod[:].rearrange("p (o i) -> p o i", i=64),
                        in0=wsb[:].rearrange("p (o i) -> p o i", i=64),
                        in1=hsfs[t][:, :].unsqueeze(1).to_broadcast([P, 64, 64]),
                        op=OP.mult,
                    )
                    # add tree over i: 64 -> 32 -> 16 -> 8 (DVE,pool), then reduce8
                    s1 = ep.tile([P, 2048], f16, name=f"s1_{d}_{t}", tag="s1")
                    pv = prod[:].rearrange("p (o i) -> p o i", i=64)
                    nc.vector.tensor_tensor(
                        out=s1[:].rearrange("p (o i) -> p o i", i=32),
                        in0=pv[:, :, 0:32], in1=pv[:, :, 32:64], op=OP.add,
                    )
                    s2 = ep.tile([P, 1024], f16, name=f"s2_{d}_{t}", tag="s2")
                    s1v = s1[:].rearrange("p (o i) -> p o i", i=32)
                    nc.gpsimd.tensor_tensor(
                        out=s2[:].rearrange("p (o i) -> p o i", i=16),
                        in0=s1v[:, :, 0:16], in1=s1v[:, :, 16:32], op=OP.add,
                    )
                    s3 = ep.tile([P, 512], f16, name=f"s3_{d}_{t}", tag="s3")
                    s2v = s2[:].rearrange("p (o i) -> p o i", i=16)
                    nc.gpsimd.tensor_tensor(
                        out=s3[:].rearrange("p (o i) -> p o i", i=8),
                        in0=s2v[:, :, 0:8], in1=s2v[:, :, 8:16], op=OP.add,
                    )
                    msg = mp.tile([P, DIM], f16, name=f"msg{d}_{t}", tag="msg")
                    with nc.allow_low_precision(reason="8-way f16 add, tol 2e-2"):
                        nc.vector.tensor_reduce(
                            out=msg[:],
                            in_=s3[:].rearrange("p (o i) -> p o i", i=8),
                            axis=mybir.AxisListType.X,
                            op=OP.add,
                        )
                    msgs.append(msg)

                # deferred scatter: PE streams all We matmuls above first
                aggT = pagg.tile([64, NPC], f32, name=f"aggT{d}", tag="agg")
                for t in range(T):
                    for s in range(2):
                        nc.tensor.matmul(
                            aggT[0:64, s * 512:(s + 1) * 512],
                            msgs[t][:],
                            S[:, t * NPC + s * 512: t * NPC + (s + 1) * 512],
                            start=(t == 0), stop=False,
                        )
                for s in range(2):
                    nc.tensor.matmul(
                        aggT[0:64, s * 512:(s + 1) * 512],
                        rootw[d][:],
                        hT16[:, s * 512:(s + 1) * 512],
                        start=False, stop=True,
                    )
                nc.scalar.activation(xch[0:64, :], aggT[0:64, :], AF.Relu, bias=convb[d][:, 0:1])

                # ---- GRU(xc, h) ----
                rz_ps = pwe.tile([P, NPC], f32, name=f"rz{d}", tag="pwe")
                mm512(lambda o, n: rz_ps[0:128, o:o + n], grzT[d][:],
                      lambda o, n: xch[:, o:o + n], NPC, True, True)
                gin_ps = pwe.tile([P, NPC], f32, name=f"gin{d}", tag="pwe")
                mm512(lambda o, n: gin_ps[0:64, o:o + n], wnx[d][:],
                      lambda o, n: xch[0:64, o:o + n], NPC, True, True)
                ghn_ps = pwe.tile([P, NPC], f32, name=f"ghn{d}", tag="pwe")
                mm512(lambda o, n: ghn_ps[0:64, o:o + n], wnh[d][:],
                      lambda o, n: hT16[:, o:o + n], NPC, True, True)

                r_s = wp.tile([64, NPC], f32, name=f"r{d}", tag="gru_r")
                nc.scalar.activation(r_s[:], rz_ps[0:64, :], AF.Sigmoid, bias=brz[d][0:64, 0:1])
                z_s = wp.tile([64, NPC], f32, name=f"z{d}", tag="gru_z")
                nc.scalar.activation(z_s[:], rz_ps[64:128, :], AF.Sigmoid, bias=brz[d][64:128, 0:1])
                gin = wp.tile([64, NPC], f32, name=f"gin{d}s", tag="gru_gin")
                nc.scalar.activation(gin[:], gin_ps[0:64, :], AF.Identity, bias=bihn[d][:, 0:1])
                hn = wp.tile([64, NPC], f32, name=f"hn{d}s", tag="gru_hn")
                nc.scalar.activation(hn[:], ghn_ps[0:64, :], AF.Identity, bias=bhhn[d][:, 0:1])

                t1 = wp.tile([64, NPC], f32, name=f"t1_{d}", tag="gru_t1")
                nc.vector.tensor_tensor(out=t1[:], in0=r_s[:], in1=hn[:], op=OP.mult)
                nc.vector.tensor_tensor(out=t1[:], in0=t1[:], in1=gin[:], op=OP.add)
                nt = wp.tile([64, NPC], f32, name=f"nt_{d}", tag="gru_nt")
                nc.scalar.activation(nt[:], t1[:], AF.Tanh)
                hm = wp.tile([64, NPC], f32, name=f"hm_{d}", tag="gru_hm")
                nc.vector.tensor_tensor(out=hm[:], in0=h_T[:], in1=nt[:], op=OP.subtract)
                nc.vector.tensor_tensor(out=hm[:], in0=hm[:], in1=z_s[:], op=OP.mult)
                h_new = sp.tile([64, NPC], f32, name=f"h_{d}", tag="hT")
                nc.vector.tensor_tensor(out=h_new[:], in0=hm[:], in1=nt[:], op=OP.add)
                h_T = h_new
                hT16 = sp.tile([64, NPC], f16, name=f"hT16_{d}", tag="hT16")
                nc.vector.tensor_copy(hT16[:], h_T[:])

                if d < DEPTHS - 1:
                    boundary(d + 1, h_T, hT16)
                else:
                    h_nm = []
                    for c in range(8):
                        tp = ptp.tile([P, DIM], f32, name=f"tp_f{c}", tag="tp")
                        nc.tensor.transpose(
                            out=tp[0:P, 0:DIM], in_=h_T[:, c * P:(c + 1) * P],
                            identity=ident[0:64, 0:64],
                        )
                        hm2 = wp.tile([P, DIM], f16, name=f"hnm_f{c}", tag=f"hnm{c}")
                        nc.scalar.activation(hm2[:], tp[:], AF.Copy)
                        h_nm.append(hm2)
                    pooled_ps = ptp.tile([64, N_GRAPHS], f32, name="pooled_ps", tag="tp")
                    for c in range(8):
                        nc.tensor.matmul(
                            pooled_ps[0:64, :],
                            h_nm[c][:],
                            pS[:, c * N_GRAPHS:(c + 1) * N_GRAPHS],
                            start=(c == 0), stop=(c == 7),
                        )
                    pooled_sb = wp.tile([64, N_GRAPHS], f32, name="pooled_sb")
                    nc.scalar.activation(pooled_sb[:], pooled_ps[0:64, :], AF.Copy)
                    nc.sync.dma_start(ar_in[:, :], pooled_sb[:])

            # ---------------- pooling AllReduce + output MLP ----------------
            nc.gpsimd.collective_compute(
                "AllReduce", OP.add, replica_groups=RG,
                ins=[ar_in.opt()], outs=[ar_out.opt()],
            )
            pooled = wp.tile([64, N_GRAPHS], f16, name="pooled")
            nc.gpsimd.dma_start(pooled[:], ar_out[:, :])

            m1_ps = ptp.tile([64, N_GRAPHS], f32, name="m1_ps", tag="tp")
            nc.tensor.matmul(m1_ps[0:64, :], o0wT[:], pooled[:], start=True, stop=True)
            m1r = wp.tile([64, N_GRAPHS], f16, name="m1r")
            nc.scalar.activation(m1r[:], m1_ps[0:64, :], AF.Relu, bias=o0b[:, 0:1])

            m2_ps = ptp.tile([64, N_GRAPHS], f32, name="m2_ps", tag="tp")
            nc.tensor.matmul(m2_ps[0:32, :], o1wT[:], m1r[:], start=True, stop=True)
            m2b = wp.tile([32, N_GRAPHS], f16, name="m2b")
            nc.scalar.activation(m2b[:], m2_ps[0:32, :], AF.Identity, bias=o1b[:, 0:1])

            m3_ps = ptp.tile([64, N_GRAPHS], f32, name="m3_ps", tag="tp")
            nc.tensor.matmul(m3_ps[0:1, :], o2wT[:], m2b[:], start=True, stop=True)
            ysb = wp.tile([1, N_GRAPHS], f32, name="ysb")
            nc.scalar.activation(ysb[:], m3_ps[0:1, :], AF.Identity, bias=o2b[:, 0:1])
            nc.sync.dma_start(y_d[:, :], ysb[:])

    nc.finalize()
    return nc


def _prep(inputs):
    g = lambda k: np.asarray(inputs[k])
    x = g("x").astype(np.float32)
    ea = g("edge_attr").astype(np.float32)
    ei = g("edge_index").astype(np.int64)
    batch = g("batch").astype(np.int64)
    src, dst = ei[0], ei[1]

    owner = dst // NPC
    core_ids = [np.nonzero(owner == c)[0] for c in range(NC)]
    T = int(max((len(ids) + P - 1) // P for ids in core_ids))
    T = max(T, 1)
    EP = T * P

    cnt = np.bincount(batch, minlength=N_GRAPHS).astype(np.float32)
    inv = 1.0 / np.maximum(cnt, 1.0)

    mlp2_b = g("mlp2_b").astype(np.float32)
    b2_zero = bool(np.all(mlp2_b == 0))

    shared = {
        "fc0_wT": g("fc0_w").astype(np.float16).T.copy(),
        "fc0_b": g("fc0_b").astype(np.float32)[:, None],
        "g0_wihT": g("gru0_wih").astype(np.float16).T.copy(),
        "g0_brz": (g("gru0_bih") + g("gru0_bhh")).astype(np.float32)[:128, None],
        "g0_bihn": g("gru0_bih").astype(np.float32)[128:, None],
        "g0_bhhn": g("gru0_bhh").astype(np.float32)[128:, None],
        "o0wT": g("out0_w").astype(np.float16).T.copy(),
        "o0b": g("out0_b").astype(np.float32)[:, None],
        "o1wT": g("out1_w").astype(np.float16).T.copy(),
        "o1b": g("out1_b").astype(np.float32)[:, None],
        "o2wT": g("out2_w").astype(np.float16).T.copy(),
        "o2b": g("out2_b").astype(np.float32)[:, None],
    }
    mlp1_w = g("mlp1_w").astype(np.float32)
    mlp1_b = g("mlp1_b").astype(np.float32)
    mlp2_w = g("mlp2_w").astype(np.float32)
    root_w = g("root_w").astype(np.float32)
    conv_b = g("conv_b").astype(np.float32)
    gru_wih = g("gru_wih").astype(np.float32)
    gru_whh = g("gru_whh").astype(np.float32)
    gru_bih = g("gru_bih").astype(np.float32)
    gru_bhh = g("gru_bhh").astype(np.float32)
    for d in range(DEPTHS):
        shared[f"w2p{d}"] = (
            mlp2_w[d].reshape(64, 64, 128).transpose(2, 1, 0).reshape(128, 4096)
        ).astype(np.float16)
        shared[f"m1wT{d}"] = mlp1_w[d].T.astype(np.float16).copy()
        shared[f"m1b{d}"] = mlp1_b[d][:, None].copy()
        shared[f"root{d}"] = root_w[d].astype(np.float16).copy()
        shared[f"convb{d}"] = conv_b[d][:, None].copy()
        shared[f"grzT{d}"] = np.concatenate(
            [gru_wih[d].T[:, 0:128], gru_whh[d].T[:, 0:128]], axis=0
        ).astype(np.float16).copy()
        shared[f"wnx{d}"] = gru_wih[d].T[:, 128:192].astype(np.float16).copy()
        shared[f"wnh{d}"] = gru_whh[d].T[:, 128:192].astype(np.float16).copy()
        shared[f"brz{d}"] = (gru_bih[d] + gru_bhh[d])[:128, None].copy()
        shared[f"bihn{d}"] = gru_bih[d][128:, None].copy()
        shared[f"bhhn{d}"] = gru_bhh[d][128:, None].copy()
        if not b2_zero:
            b2p = mlp2_b[d].reshape(64, 64).T.reshape(4096)
            shared[f"b2bc{d}"] = np.broadcast_to(
                b2p.astype(np.float16), (P, 4096)
            ).copy()

    in_maps = []
    for c in range(NC):
        ids = core_ids[c]
        n_real = len(ids)
        src_pad = np.zeros(EP, np.int32)
        src_pad[:n_real] = src[ids]
        ea_pad = np.zeros((EP, 10), np.float32)
        ea_pad[:n_real] = ea[ids]
        S_full = np.zeros((EP, NPC), np.float16)
        S_full[np.arange(n_real), dst[ids] - c * NPC] = 1.0
        S_tab = np.zeros((P, T * NPC), np.float16)
        for t in range(T):
            S_tab[:, t * NPC:(t + 1) * NPC] = S_full[t * P:(t + 1) * P]
        pm = np.zeros((NPC, N_GRAPHS), np.float16)
        nb = batch[c * NPC:(c + 1) * NPC]
        pm[np.arange(NPC), nb] = inv[nb].astype(np.float16)
        m = {
            "xT": x[c * NPC:(c + 1) * NPC].T.astype(np.float16).copy(),
            "eaT": ea_pad.T.astype(np.float16).copy(),
            "srcidx": src_pad.reshape(T, P).T.copy(),
            "S": S_tab,
            "poolS": pm,
        }
        m.update(shared)
        in_maps.append(m)
    return T, b2_zero, in_maps


def kernel(**inputs) -> np.ndarray:
    global LAST_EXEC_NS, LAST_RESULTS
    T, b2_zero, in_maps = _prep(inputs)
    key = (T, b2_zero)
    if key not in _CACHE:
        _CACHE[key] = _build(T, b2_zero)
    nc = _CACHE[key]

    from concourse.bass_utils import run_bass_kernel_spmd

    if TRACE:
        res = run_bass_kernel_spmd(
            nc, in_maps, list(range(NC)), trace=True, trace_cores=list(range(NC))
        )
        LAST_EXEC_NS = res.exec_time_ns
        LAST_RESULTS = res
    else:
        res = run_bass_kernel_spmd(nc, in_maps, list(range(NC)))
    return res.results[0]["y"].reshape(N_GRAPHS).astype(np.float32)


# revision 8
# speedup vs baseline: 1.2177x; 1.1329x over previous
"""NNConv+GRU message-passing network (ConvGRU) on 8 Trainium2 NeuronCores.

Strategy (v2):
  - Edges sharded by OWNER OF DST node (8 ranges of 1024 nodes). Each core
    computes edge MLP + per-edge matvec + scatter-add locally; scatter is a
    matmul against a static 0/1 selection matrix (exact duplicate handling).
  - Node state h node-sharded for the GRU; h exchanged across cores in f16
    via AllGather once per conv layer; per-edge h[src] gathered by indirect
    DMA directly in f16.
  - All matmuls run in fp16 (fp32 matmuls cost 4 cyc/col on TRN2 PE).
  - Per-edge weight matrices We never touch DRAM: PE -> PSUM f32, ACT
    evacuates to SBUF f16, DVE does one broadcast-multiply per tile (2x_1p
    fast mode) and a packed-halves ADD TREE (tensor_tensor 2x) instead of
    tensor_reduce (which has no DVE fast modes); two mid tree levels run on
    GpSimd to balance engine load.
  - Scatter matmuls are deferred to the end of each depth so PE streams all
    We matmuls back to back (p-state ramp, no per-tile PE<->DVE lockstep).
  - GRU r/z gates computed by ONE stacked matmul over [xc; h] (k=128).

Self-contained: numpy + concourse/bass only. Shapes hardcoded.
"""
import numpy as np

DIM = 64
DEPTHS = 3
N_NODES = 8192
N_EDGES = 16384
N_GRAPHS = 64
NC = 8
NPC = N_NODES // NC   # 1024 nodes per core
P = 128

TRACE = False
LAST_EXEC_NS = None
LAST_RESULTS = None

_CACHE = {}


def _build(T, b2_zero):
    import concourse.mybir as mybir
    import concourse.tile as tile
    from concourse import bacc
    import concourse.bass as bass
    from concourse.masks import make_identity

    f32 = mybir.dt.float32
    f16 = mybir.dt.float16
    i32 = mybir.dt.int32
    AF = mybir.ActivationFunctionType
    OP = mybir.AluOpType
    EP = T * P

    nc = bacc.Bacc("TRN2", target_bir_lowering=False, debug=False, num_devices=NC)

    def din(name, shape, dt=f32):
        return nc.dram_tensor(name, shape, dt, kind="ExternalInput")

    xT_d = din("xT", [40, NPC], f16)
    eaT_d = din("eaT", [10, EP], f16)
    srcx_d = din("srcidx", [P, T], i32)
    S_d = din("S", [P, T * NPC], f16)
    pS_d = din("poolS", [P, 8 * N_GRAPHS], f16)
    NB32 = 22
    NB16 = 1665
    wb32_d = din("wb32", [P, NB32])
    wb16_d = din("wb16", [P, NB16], f16)
    w2p_d = [din(f"w2p{d}", [128, 4096], f16) for d in range(DEPTHS)]
    b2bc_d = None if b2_zero else [din(f"b2bc{d}", [128, 4096], f16) for d in range(DEPTHS)]

    y_d = nc.dram_tensor("y", [1, N_GRAPHS], f32, kind="ExternalOutput")

    RG = [list(range(NC))]

    with tile.TileContext(nc) as tc:
        with (
            tc.tile_pool(name="const", bufs=1) as cp,
            tc.tile_pool(name="work", bufs=1) as wp,      # gru work tiles
            tc.tile_pool(name="state", bufs=2) as sp,     # h state tiles
            tc.tile_pool(name="edge", bufs=2) as ep,      # big per-tile buffers
            tc.tile_pool(name="gath", bufs=6) as gp,      # gathered h[src]
            tc.tile_pool(name="msgp", bufs=T + 1) as mp,  # per-tile msg outputs
            tc.tile_pool(name="pwe", bufs=2, space="PSUM") as pwe,
            tc.tile_pool(name="pagg", bufs=1, space="PSUM") as pagg,
            tc.tile_pool(name="ptp", bufs=2, space="PSUM") as ptp,
            tc.tile_pool(name="dram", bufs=1, space="DRAM") as dp,
        ):
            # ---------------- constants to SBUF ----------------
            def load(name, dram, shape, dt=f32, ap=None):
                t = cp.tile(shape, dt, name=name)
                nc.sync.dma_start(t[:], dram[:, :] if ap is None else ap)
                return t

            # need-ordered loads: small blobs first, then per-use bigs
            wb32 = load("wb32_s", wb32_d, [P, 22])
            wb16 = load("wb16_s", wb16_d, [P, 1665], f16)
            xT = load("xT_s", xT_d, [40, NPC], f16)
            srcx = load("srcx_s", srcx_d, [P, T], i32)
            eaT = load("eaT_s", eaT_d, [10, EP], f16)
            w2p = [load(f"w2p_s{d}", w2p_d[d], [128, 4096], f16) for d in range(DEPTHS)]
            S = load("S_s", S_d, [P, T * NPC], f16)
            pS = load("pS_s", pS_d, [P, 8 * N_GRAPHS], f16)
            b2bc = (
                None if b2_zero else
                [load(f"b2bc_s{d}", b2bc_d[d], [128, 4096], f16) for d in range(DEPTHS)]
            )

            # blob slicing (column layout must match _prep)
            c32 = iter(range(22))
            def b32(rows):
                j = next(c32)
                return wb32[0:rows, j:j + 1]
            fc0_b = b32(32)
            g0_brz = b32(128)
            g0_bihn = b32(64)
            g0_bhhn = b32(64)
            m1b = [b32(128) for _ in range(DEPTHS)]
            convb = [b32(64) for _ in range(DEPTHS)]
            brz = [b32(128) for _ in range(DEPTHS)]
            bihn = [b32(64) for _ in range(DEPTHS)]
            bhhn = [b32(64) for _ in range(DEPTHS)]
            o0b = b32(64)
            o1b = b32(32)
            o2b = b32(1)

            _c16 = [0]
            def b16(rows, cols):
                j = _c16[0]
                _c16[0] += cols
                return wb16[0:rows, j:j + cols]
            fc0_wT = b16(40, 32)
            g0_wihT = b16(32, 192)
            grzT = [b16(128, 128) for _ in range(DEPTHS)]
            wnx = [b16(64, 64) for _ in range(DEPTHS)]
            wnh = [b16(64, 64) for _ in range(DEPTHS)]
            rootw = [b16(64, 64) for _ in range(DEPTHS)]
            m1wT = [b16(10, 128) for _ in range(DEPTHS)]
            o0wT = b16(64, 64)
            o1wT = b16(64, 32)
            o2wT = b16(32, 1)

            ident = cp.tile([P, P], f32, name="ident")
            make_identity(nc, ident[:])

            hown = [dp.tile([NPC, DIM], f16, name=f"hown{d}") for d in range(DEPTHS)]
            hfull = [dp.tile([N_NODES, DIM], f16, name=f"hfull{d}") for d in range(DEPTHS)]
            ar_in = dp.tile([DIM, N_GRAPHS], f32, name="ar_in")
            ar_out = dp.tile([DIM, N_GRAPHS], f32, name="ar_out")

            # ---------------- helpers ----------------
            def mm512(out_ap_fn, lhsT, rhs_fn, n_total, start, stop):
                off = 0
                while off < n_total:
                    n = min(512, n_total - off)
                    nc.tensor.matmul(
                        out_ap_fn(off, n), lhsT, rhs_fn(off, n),
                        start=start, stop=stop,
                    )
                    off += n

            def boundary(d_next, h_T, hT16):
                """h_T f32 [64,1024] -> node-major f16 tiles, DRAM store,
                AllGather into hfull[d_next]; also fills xch[d] bottom half
                lazily via the hT16 tile the caller made."""
                outs = []
                for c in range(8):
                    tp = ptp.tile([P, DIM], f32, name=f"tp_b{d_next}_{c}", tag="tp")
                    nc.tensor.transpose(
                        out=tp[0:P, 0:DIM], in_=h_T[:, c * P:(c + 1) * P],
                        identity=ident[0:64, 0:64],
                    )
                    hm = wp.tile([P, DIM], f16, name=f"hnm_{d_next}_{c}", tag=f"hnm{c}")
                    nc.scalar.activation(hm[:], tp[:], AF.Copy)
                    outs.append(hm)
                for c in range(8):
                    nc.sync.dma_start(hown[d_next][c * P:(c + 1) * P, :], outs[c][:])
                nc.gpsimd.collective_compute(
                    "AllGather", OP.bypass, replica_groups=RG,
                    ins=[hown[d_next].opt()], outs=[hfull[d_next].opt()],
                )
                return outs

            # ---------------- phase 0: fc0 + gru0 (h0 = 0) ----------------
            x0_ps = pwe.tile([P, NPC], f32, name="x0_ps", tag="pwe")
            mm512(lambda o, n: x0_ps[0:32, o:o + n], fc0_wT[:],
                  lambda o, n: xT[:, o:o + n], NPC, True, True)
            x0r = wp.tile([32, NPC], f16, name="x0r")
            nc.scalar.activation(x0r[:], x0_ps[0:32, :], AF.Relu, bias=fc0_b[:, 0:1])

            # edge-MLP hidden for ALL depths up front (h-independent)
            hidT = []
            for d in range(DEPTHS):
                ht = cp.tile([P, EP], f16, name=f"hidT{d}")
                off = 0
                while off < EP:
                    n = min(1024, EP - off)
                    hp = pwe.tile([P, NPC], f32, name=f"hid_ps{d}_{off}", tag="pwe")
                    mm512(lambda o, nn, _b=off: hp[:, o:o + nn], m1wT[d][:],
                          lambda o, nn, _b=off: eaT[:, _b + o:_b + o + nn], n, True, True)
                    nc.scalar.activation(
                        ht[:, off:off + n], hp[:, 0:n], AF.Relu, bias=m1b[d][:, 0:1]
                    )
                    off += n
                hidT.append(ht)

            g0rz_ps = pwe.tile([P, NPC], f32, name="g0rz_ps", tag="pwe")
            mm512(lambda o, n: g0rz_ps[0:128, o:o + n], g0_wihT[:, 0:128],
                  lambda o, n: x0r[:, o:o + n], NPC, True, True)
            r0 = wp.tile([64, NPC], f32, name="r0", tag="gru_r")
            nc.scalar.activation(r0[:], g0rz_ps[0:64, :], AF.Sigmoid, bias=g0_brz[0:64, 0:1])
            z0 = wp.tile([64, NPC], f32, name="z0", tag="gru_z")
            nc.scalar.activation(z0[:], g0rz_ps[64:128, :], AF.Sigmoid, bias=g0_brz[64:128, 0:1])

            g0n_ps = pwe.tile([P, NPC], f32, name="g0n_ps", tag="pwe")
            mm512(lambda o, n: g0n_ps[0:64, o:o + n], g0_wihT[:, 128:192],
                  lambda o, n: x0r[:, o:o + n], NPC, True, True)
            gin0 = wp.tile([64, NPC], f32, name="gin0", tag="gru_gin")
            nc.scalar.activation(gin0[:], g0n_ps[0:64, :], AF.Identity, bias=g0_bihn[:, 0:1])

            # n = tanh(gi_n + bih_n + r*bhh_n); h' = n - z*n   (h0 = 0)
            t10 = wp.tile([64, NPC], f32, name="t10", tag="gru_t1")
            nc.vector.tensor_scalar_mul(t10[:], r0[:], g0_bhhn[:, 0:1])
            nc.vector.tensor_tensor(out=t10[:], in0=t10[:], in1=gin0[:], op=OP.add)
            nt0 = wp.tile([64, NPC], f32, name="nt0", tag="gru_nt")
            nc.scalar.activation(nt0[:], t10[:], AF.Tanh)
            hm0 = wp.tile([64, NPC], f32, name="hm0", tag="gru_hm")
            nc.vector.tensor_tensor(out=hm0[:], in0=z0[:], in1=nt0[:], op=OP.mult)
            h_T = sp.tile([64, NPC], f32, name="h_p0", tag="hT")
            nc.vector.tensor_tensor(out=h_T[:], in0=nt0[:], in1=hm0[:], op=OP.subtract)

            hT16 = sp.tile([64, NPC], f16, name="hT16_p0", tag="hT16")
            nc.vector.tensor_copy(hT16[:], h_T[:])
            boundary(0, h_T, hT16)

            # ---------------- conv depths ----------------
            for d in range(DEPTHS):
                # gathers first (gpsimd queue: AG(d) already issued before these)
                hsfs = []
                for t in range(T):
                    hsf = gp.tile([P, DIM], f16, name=f"hsf{d}_{t}", tag="hsf")
                    nc.gpsimd.indirect_dma_start(
                        out=hsf[:], out_offset=None,
                        in_=hfull[d][:, :],
                        in_offset=bass.IndirectOffsetOnAxis(ap=srcx[:, t:t + 1], axis=0),
                    )
                    hsfs.append(hsf)

                # xch: [xc ; h] stacked moving operand for GRU matmuls
                xch = sp.tile([P, NPC], f16, name=f"xch{d}", tag="xch")
                nc.sync.dma_start(xch[64:128, :], hT16[:])

                msgs = []
                pend = []

                def finish_tile(s2, t):
                    s3 = ep.tile([P, 512], f16, name=f"s3_{d}_{t}", tag="s3")
                    s2v = s2[:].rearrange("p (o i) -> p o i", i=16)
                    nc.vector.tensor_tensor(
                        out=s3[:].rearrange("p (o i) -> p o i", i=8),
                        in0=s2v[:, :, 0:8], in1=s2v[:, :, 8:16], op=OP.add,
                    )
                    msg = mp.tile([P, DIM], f16, name=f"msg{d}_{t}", tag="msg")
                    with nc.allow_low_precision(reason="8-way f16 add, tol 2e-2"):
                        nc.vector.tensor_reduce(
                            out=msg[:],
                            in_=s3[:].rearrange("p (o i) -> p o i", i=8),
                            axis=mybir.AxisListType.X,
                            op=OP.add,
                        )
                    msgs.append(msg)

                for t in range(T):
                    wsb = ep.tile([P, 4096], f16, name=f"wsb{d}_{t}", tag="wsb")
                    for q in range(4):
                        wps = pwe.tile([P, NPC], f32, name=f"we{d}_{t}_{q}", tag="pwe")
                        mm512(lambda o, n, _q=q: wps[:, o:o + n],
                              hidT[d][:, t * P:(t + 1) * P],
                              lambda o, n, _q=q: w2p[d][:, _q * 1024 + o:_q * 1024 + o + n],
                              1024, True, True)
                        nc.scalar.activation(
                            wsb[:, q * 1024:(q + 1) * 1024], wps[:], AF.Copy
                        )
                    if b2bc is not None:
                        nc.vector.tensor_tensor(
                            out=wsb[:], in0=wsb[:], in1=b2bc[d][:], op=OP.add
                        )
                    # prod[e, (o,i)] = wsb * hsrc[e,i] ; one whole-tile DVE op
                    prod = ep.tile([P, 4096], f16, name=f"prod{d}_{t}", tag="prod")
                    nc.vector.tensor_tensor(
                        out=prod[:].rearrange("p (o i) -> p o i", i=64),
                        in0=wsb[:].rearrange("p (o i) -> p o i", i=64),
                        in1=hsfs[t][:, :].unsqueeze(1).to_broadcast([P, 64, 64]),
                        op=OP.mult,
                    )
                    # tree over i: L1 (DVE), L2 (Pool), then deferred L3+reduce8
                    # (DVE) issued one tile later so DVE never waits on Pool.
                    s1 = ep.tile([P, 2048], f16, name=f"s1_{d}_{t}", tag="s1")
                    pv = prod[:].rearrange("p (o i) -> p o i", i=64)
                    nc.vector.tensor_tensor(
                        out=s1[:].rearrange("p (o i) -> p o i", i=32),
                        in0=pv[:, :, 0:32], in1=pv[:, :, 32:64], op=OP.add,
                    )
                    s2 = ep.tile([P, 1024], f16, name=f"s2_{d}_{t}", tag="s2", bufs=3)
                    s1v = s1[:].rearrange("p (o i) -> p o i", i=32)
                    nc.gpsimd.tensor_tensor(
                        out=s2[:].rearrange("p (o i) -> p o i", i=16),
                        in0=s1v[:, :, 0:16], in1=s1v[:, :, 16:32], op=OP.add,
                    )
                    pend.append((s2, t))
                    if len(pend) > 1:
                        finish_tile(*pend.pop(0))
                while pend:
                    finish_tile(*pend.pop(0))

                # deferred scatter: PE streams all We matmuls above first
                aggT = pagg.tile([64, NPC], f32, name=f"aggT{d}", tag="agg")
                for t in range(T):
                    for s in range(2):
                        nc.tensor.matmul(
                            aggT[0:64, s * 512:(s + 1) * 512],
                            msgs[t][:],
                            S[:, t * NPC + s * 512: t * NPC + (s + 1) * 512],
                            start=(t == 0), stop=False,
                        )
                for s in range(2):
                    nc.tensor.matmul(
                        aggT[0:64, s * 512:(s + 1) * 512],
                        rootw[d][:],
                        hT16[:, s * 512:(s + 1) * 512],
                        start=False, stop=True,
                    )
                nc.scalar.activation(xch[0:64, :], aggT[0:64, :], AF.Relu, bias=convb[d][:, 0:1])

                # ---- GRU(xc, h) ----
                rz_ps = pwe.tile([P, NPC], f32, name=f"rz{d}", tag="pwe")
                mm512(lambda o, n: rz_ps[0:128, o:o + n], grzT[d][:],
                      lambda o, n: xch[:, o:o + n], NPC, True, True)
                gin_ps = pwe.tile([P, NPC], f32, name=f"gin{d}", tag="pwe")
                mm512(lambda o, n: gin_ps[0:64, o:o + n], wnx[d][:],
                      lambda o, n: xch[0:64, o:o + n], NPC, True, True)
                ghn_ps = pwe.tile([P, NPC], f32, name=f"ghn{d}", tag="pwe")
                mm512(lambda o, n: ghn_ps[0:64, o:o + n], wnh[d][:],
                      lambda o, n: hT16[:, o:o + n], NPC, True, True)

                r_s = wp.tile([64, NPC], f32, name=f"r{d}", tag="gru_r")
                nc.scalar.activation(r_s[:], rz_ps[0:64, :], AF.Sigmoid, bias=brz[d][0:64, 0:1])
                z_s = wp.tile([64, NPC], f32, name=f"z{d}", tag="gru_z")
                nc.scalar.activation(z_s[:], rz_ps[64:128, :], AF.Sigmoid, bias=brz[d][64:128, 0:1])
                gin = wp.tile([64, NPC], f32, name=f"gin{d}s", tag="gru_gin")
                nc.scalar.activation(gin[:], gin_ps[0:64, :], AF.Identity, bias=bihn[d][:, 0:1])
                hn = wp.tile([64, NPC], f32, name=f"hn{d}s", tag="gru_hn")
                nc.scalar.activation(hn[:], ghn_ps[0:64, :], AF.Identity, bias=bhhn[d][:, 0:1])

                t1 = wp.tile([64, NPC], f32, name=f"t1_{d}", tag="gru_t1")
                nc.vector.tensor_tensor(out=t1[:], in0=r_s[:], in1=hn[:], op=OP.mult)
                nc.vector.tensor_tensor(out=t1[:], in0=t1[:], in1=gin[:], op=OP.add)
                nt = wp.tile([64, NPC], f32, name=f"nt_{d}", tag="gru_nt")
                nc.scalar.activation(nt[:], t1[:], AF.Tanh)
                hm = wp.tile([64, NPC], f32, name=f"hm_{d}", tag="gru_hm")
                nc.vector.tensor_tensor(out=hm[:], in0=h_T[:], in1=nt[:], op=OP.subtract)
                nc.vector.tensor_tensor(out=hm[:], in0=hm[:], in1=z_s[:], op=OP.mult)
                h_new = sp.tile([64, NPC], f32, name=f"h_{d}", tag="hT")
                nc.vector.tensor_tensor(out=h_new[:], in0=hm[:], in1=nt[:], op=OP.add)
                h_T = h_new
                hT16 = sp.tile([64, NPC], f16, name=f"hT16_{d}", tag="hT16")
                nc.vector.tensor_copy(hT16[:], h_T[:])

                if d < DEPTHS - 1:
                    boundary(d + 1, h_T, hT16)
                else:
                    h_nm = []
                    for c in range(8):
                        tp = ptp.tile([P, DIM], f32, name=f"tp_f{c}", tag="tp")
                        nc.tensor.transpose(
                            out=tp[0:P, 0:DIM], in_=h_T[:, c * P:(c + 1) * P],
                            identity=ident[0:64, 0:64],
                        )
                        hm2 = wp.tile([P, DIM], f16, name=f"hnm_f{c}", tag=f"hnm{c}")
                        nc.scalar.activation(hm2[:], tp[:], AF.Copy)
                        h_nm.append(hm2)
                    pooled_ps = ptp.tile([64, N_GRAPHS], f32, name="pooled_ps", tag="tp")
                    for c in range(8):
                        nc.tensor.matmul(
                            pooled_ps[0:64, :],
                            h_nm[c][:],
                            pS[:, c * N_GRAPHS:(c + 1) * N_GRAPHS],
                            start=(c == 0), stop=(c == 7),
                        )
                    pooled_sb = wp.tile([64, N_GRAPHS], f32, name="pooled_sb")
                    nc.scalar.activation(pooled_sb[:], pooled_ps[0:64, :], AF.Copy)
                    nc.sync.dma_start(ar_in[:, :], pooled_sb[:])

            # ---------------- pooling AllReduce + output MLP ----------------
            nc.gpsimd.collective_compute(
                "AllReduce", OP.add, replica_groups=RG,
                ins=[ar_in.opt()], outs=[ar_out.opt()],
            )
            pooled = wp.tile([64, N_GRAPHS], f16, name="pooled")
            nc.gpsimd.dma_start(pooled[:], ar_out[:, :])

            m1_ps = ptp.tile([64, N_GRAPHS], f32, name="m1_ps", tag="tp")
            nc.tensor.matmul(m1_ps[0:64, :], o0wT[:], pooled[:], start=True, stop=True)
            m1r = wp.tile([64, N_GRAPHS], f16, name="m1r")
            nc.scalar.activation(m1r[:], m1_ps[0:64, :], AF.Relu, bias=o0b[:, 0:1])

            m2_ps = ptp.tile([64, N_GRAPHS], f32, name="m2_ps", tag="tp")
            nc.tensor.matmul(m2_ps[0:32, :], o1wT[:], m1r[:], start=True, stop=True)
            m2b = wp.tile([32, N_GRAPHS], f16, name="m2b")
            nc.scalar.activation(m2b[:], m2_ps[0:32, :], AF.Identity, bias=o1b[:, 0:1])

            m3_ps = ptp.tile([64, N_GRAPHS], f32, name="m3_ps", tag="tp")
            nc.tensor.matmul(m3_ps[0:1, :], o2wT[:], m2b[:], start=True, stop=True)
            ysb = wp.tile([1, N_GRAPHS], f32, name="ysb")
            nc.scalar.activation(ysb[:], m3_ps[0:1, :], AF.Identity, bias=o2b[:, 0:1])
            nc.sync.dma_start(y_d[:, :], ysb[:])

    nc.finalize()
    return nc


def _prep(inputs):
    g = lambda k: np.asarray(inputs[k])
    x = g("x").astype(np.float32)
    ea = g("edge_attr").astype(np.float32)
    ei = g("edge_index").astype(np.int64)
    batch = g("batch").astype(np.int64)
    src, dst = ei[0], ei[1]

    owner = dst // NPC
    core_ids = [np.nonzero(owner == c)[0] for c in range(NC)]
    T = int(max((len(ids) + P - 1) // P for ids in core_ids))
    T = max(T, 1)
    EP = T * P

    cnt = np.bincount(batch, minlength=N_GRAPHS).astype(np.float32)
    inv = 1.0 / np.maximum(cnt, 1.0)

    mlp2_b = g("mlp2_b").astype(np.float32)
    b2_zero = bool(np.all(mlp2_b == 0))

    mlp1_w = g("mlp1_w").astype(np.float32)
    mlp1_b = g("mlp1_b").astype(np.float32)
    mlp2_w = g("mlp2_w").astype(np.float32)
    root_w = g("root_w").astype(np.float32)
    conv_b = g("conv_b").astype(np.float32)
    gru_wih = g("gru_wih").astype(np.float32)
    gru_whh = g("gru_whh").astype(np.float32)
    gru_bih = g("gru_bih").astype(np.float32)
    gru_bhh = g("gru_bhh").astype(np.float32)

    # --- f32 bias blob [128, 22]; column order mirrors kernel b32() calls
    cols32 = []
    cols32.append(g("fc0_b").astype(np.float32))
    cols32.append((g("gru0_bih") + g("gru0_bhh")).astype(np.float32)[:128])
    cols32.append(g("gru0_bih").astype(np.float32)[128:])
    cols32.append(g("gru0_bhh").astype(np.float32)[128:])
    for d in range(DEPTHS):
        cols32.append(mlp1_b[d])
    for d in range(DEPTHS):
        cols32.append(conv_b[d])
    for d in range(DEPTHS):
        cols32.append((gru_bih[d] + gru_bhh[d])[:128])
    for d in range(DEPTHS):
        cols32.append(gru_bih[d][128:])
    for d in range(DEPTHS):
        cols32.append(gru_bhh[d][128:])
    cols32.append(g("out0_b").astype(np.float32))
    cols32.append(g("out1_b").astype(np.float32))
    cols32.append(g("out2_b").astype(np.float32))
    wb32 = np.zeros((P, len(cols32)), np.float32)
    for j, c in enumerate(cols32):
        wb32[:len(c), j] = c

    # --- f16 weight blob [128, 1665]; block order mirrors kernel b16() calls
    blocks16 = []
    blocks16.append(g("fc0_w").astype(np.float16).T)
    blocks16.append(g("gru0_wih").astype(np.float16).T)
    for d in range(DEPTHS):
        blocks16.append(np.concatenate(
            [gru_wih[d].T[:, 0:128], gru_whh[d].T[:, 0:128]], axis=0
        ).astype(np.float16))
    for d in range(DEPTHS):
        blocks16.append(gru_wih[d].T[:, 128:192].astype(np.float16))
    for d in range(DEPTHS):
        blocks16.append(gru_whh[d].T[:, 128:192].astype(np.float16))
    for d in range(DEPTHS):
        blocks16.append(root_w[d].astype(np.float16))
    for d in range(DEPTHS):
        blocks16.append(mlp1_w[d].T.astype(np.float16))
    blocks16.append(g("out0_w").astype(np.float16).T)
    blocks16.append(g("out1_w").astype(np.float16).T)
    blocks16.append(g("out2_w").astype(np.float16).T)
    ncols16 = sum(b.shape[1] for b in blocks16)
    wb16 = np.zeros((P, ncols16), np.float16)
    j = 0
    for b in blocks16:
        wb16[:b.shape[0], j:j + b.shape[1]] = b
        j += b.shape[1]

    shared = {"wb32": wb32, "wb16": wb16}
    for d in range(DEPTHS):
        shared[f"w2p{d}"] = (
            mlp2_w[d].reshape(64, 64, 128).transpose(2, 1, 0).reshape(128, 4096)
        ).astype(np.float16)
        if not b2_zero:
            b2p = mlp2_b[d].reshape(64, 64).T.reshape(4096)
            shared[f"b2bc{d}"] = np.broadcast_to(
                b2p.astype(np.float16), (P, 4096)
            ).copy()

    in_maps = []
    for c in range(NC):
        ids = core_ids[c]
        n_real = len(ids)
        src_pad = np.zeros(EP, np.int32)
        src_pad[:n_real] = src[ids]
        ea_pad = np.zeros((EP, 10), np.float32)
        ea_pad[:n_real] = ea[ids]
        S_full = np.zeros((EP, NPC), np.float16)
        S_full[np.arange(n_real), dst[ids] - c * NPC] = 1.0
        S_tab = np.zeros((P, T * NPC), np.float16)
        for t in range(T):
            S_tab[:, t * NPC:(t + 1) * NPC] = S_full[t * P:(t + 1) * P]
        pm0 = np.zeros((NPC, N_GRAPHS), np.float16)
        nb = batch[c * NPC:(c + 1) * NPC]
        pm0[np.arange(NPC), nb] = inv[nb].astype(np.float16)
        pm = np.zeros((P, 8 * N_GRAPHS), np.float16)
        for cc in range(8):
            pm[:, cc * N_GRAPHS:(cc + 1) * N_GRAPHS] = pm0[cc * P:(cc + 1) * P]
        m = {
            "xT": x[c * NPC:(c + 1) * NPC].T.astype(np.float16).copy(),
            "eaT": ea_pad.T.astype(np.float16).copy(),
            "srcidx": src_pad.reshape(T, P).T.copy(),
            "S": S_tab,
            "poolS": pm,
        }
        m.update(shared)
        in_maps.append(m)
    return T, b2_zero, in_maps


def kernel(**inputs) -> np.ndarray:
    global LAST_EXEC_NS, LAST_RESULTS
    T, b2_zero, in_maps = _prep(inputs)
    key = (T, b2_zero)
    if key not in _CACHE:
        _CACHE[key] = _build(T, b2_zero)
    nc = _CACHE[key]

    from concourse.bass_utils import run_bass_kernel_spmd

    if TRACE:
        res = run_bass_kernel_spmd(
            nc, in_maps, list(range(NC)), trace=True, trace_cores=list(range(NC))
        )
        LAST_EXEC_NS = res.exec_time_ns
        LAST_RESULTS = res
    else:
        res = run_bass_kernel_spmd(nc, in_maps, list(range(NC)))
    return res.results[0]["y"].reshape(N_GRAPHS).astype(np.float32)
